# revision 29
# baseline (speedup 1.0000x reference)
"""DSS (Diagonal State Space) layer as a Bass/Tile kernel for 8 Trainium2 NeuronCores.

Channels H sharded 8 x 128. Per core, a polyphase overlap-save FFT convolution:

  1. Forward: each 512-sample chunk c gets ONE packed 1024-point partial DFT
     A^c = CF_{c%2}^T u_c (32 PE matmuls), where CF_odd = CF_even * (-1)^f.
     Each block t then needs only the sum A^{t-1} + A^t: the (-1)^k phase of the
     second polyphase leg is folded into the odd-chunk stationaries, the filter
     parity variant, and nothing else - each u sample is forward-transformed
     once instead of twice (256 vs 480 big matmuls).
  2. Pointwise: Y = F (.) (A^{t-1}+A^t) with F = s(.)K' on even blocks, K' on odd
     (K' = K_f + D, the skip connection folded in). Sums run on GPSIMD, the
     complex multiply on DVE, all in bf16 (2x DVE rate).
  3. Inverse: one shared inverse-DFT stationary set (window [512,1024)) for all
     blocks; 8 matmuls per 128 output samples, moving operand bf16.
  4. DSS kernel k built on device: transcendental chains restacked to
     [128,64] (full partition width, re on DVE / im on GPSIMD), two-level
     power tables GW (W z^b) / Z (z^32a) by complex doubling, channel-PAIRED
     mode-sum matmuls (zero-quadrant moving operand), SEL-matmul transpose to
     kc l-major layout (no SWDGE shuffle), packed K_f DFT reusing the forward
     chunk stationaries in both parities.
"""

import sys

for _p in ("/opt/trn_rl_repo", "/opt/trn_rl_repo/concourse"):
    if _p not in sys.path:
        sys.path.insert(0, _p)

import numpy as np
from contextlib import ExitStack

import concourse.bacc as bacc
import concourse.tile as tile
import concourse.mybir as mybir

dt = mybir.dt
f32 = np.float32

B, L, H, N = 4, 4096, 1024, 64
LK = 512
F = 1024          # FFT length (overlap-save)
HOP = 512         # block hop / chunk size
NCORES = 8
HS = H // NCORES  # 128 channels per core
NBLK = L // HOP   # 8 blocks == 8 chunks
NFT = 4           # packed frequency tiles (512 freqs, Nyquist folded in sin f=0)
NJ = 4            # contraction sub-chunks per 512-sample chunk
NLT = HOP // 128  # 4 output l-tiles per block
NCH = L // 128    # 32 u sub-chunk tiles per core


# ---------------------------------------------------------------- host constants
def build_constants():
    l = np.arange(F, dtype=np.float64)[:, None]
    f = np.arange(512, dtype=np.float64)[None, :]
    ang = 2 * np.pi * l * f / F
    C = np.cos(ang)
    S = -np.sin(ang)
    S[:, 0] = (-1.0) ** np.arange(F)      # Nyquist row packed into sin-tile col 0
    sgn = (-1.0) ** np.arange(512)        # (-1)^f; f=0 (DC + packed Nyquist) -> +1
    # CF[par, cs, l', j, ft, f]: chunk-DFT stationaries (rows l' = 0..511 only),
    # one contiguous [128, 4*4*128] slab per (par, cs) -> single merged DMA
    CF = np.zeros((2, 2, 128, NJ, NFT, 128))
    for par in range(2):
        for j in range(NJ):
            for ft in range(NFT):
                cs_c = C[128 * j:128 * j + 128, 128 * ft:128 * ft + 128]
                cs_s = S[128 * j:128 * j + 128, 128 * ft:128 * ft + 128]
                if par == 1:
                    sg = sgn[None, 128 * ft:128 * ft + 128]
                    cs_c = cs_c * sg
                    cs_s = cs_s * sg
                CF[par, 0, :, j, ft, :] = cs_c
                CF[par, 1, :, j, ft, :] = cs_s
    # inverse stationaries, window [512, 1024) (shared by all blocks)
    lc = 512 + np.arange(512, dtype=np.float64)[None, :]
    fr = np.arange(512, dtype=np.float64)[:, None]
    cf_ = np.where(fr == 0, 1.0, 2.0)
    Ar = cf_ * np.cos(2 * np.pi * fr * lc / F) / F
    Ai = -(2.0 / F) * np.sin(2 * np.pi * fr * lc / F)
    Ai[0, :] = ((-1.0) ** lc[0]) / F      # Nyquist inverse row
    AI = np.zeros((2, 128, NFT, NLT, 128))   # (cs, f_low, ft, lt, l)
    for ft in range(NFT):
        for lt in range(NLT):
            AI[0, :, ft, lt, :] = Ar[128 * ft:128 * ft + 128, 128 * lt:128 * lt + 128]
            AI[1, :, ft, lt, :] = Ai[128 * ft:128 * ft + 128, 128 * lt:128 * lt + 128]
    # SGN row for the sign-flipped D outer product
    SGN = sgn.reshape(NFT, 128)[0:1, :].copy()   # (-1)^f pattern repeats per ft tile
    # SEL[b, al, l]: SEL[b, al, 32*al+b] = 1  (kc partition placement)
    SEL = np.zeros((32, 4, 128))
    for al in range(4):
        for b in range(32):
            SEL[b, al, 32 * al + b] = 1.0
    import ml_dtypes
    bf16 = ml_dtypes.bfloat16
    return CF.astype(bf16), AI.astype(bf16), SGN.astype(f32), SEL.astype(f32)


# Horner coefficient lists (highest degree first)
def _fact(k):
    r = 1.0
    for i in range(2, k + 1):
        r *= i
    return r


EXP8 = [1.0 / _fact(k) for k in range(8, -1, -1)]            # e^x, |x| <~ 0.9
EXP7 = [1.0 / _fact(k) for k in range(7, -1, -1)]            # e^x, |x| <~ 0.3
SIN9 = [1.0 / _fact(9), -1.0 / _fact(7), 1.0 / _fact(5), -1.0 / _fact(3), 1.0]
COSC = [1.0 / _fact(10), -1.0 / _fact(8), 1.0 / _fact(6), -1.0 / _fact(4),
        1.0 / _fact(2)]


class _Prog:
    def __init__(self):
        self.nc = None
        self.built = False


_prog = _Prog()


def _emit_kernel(nc, tc, ctx, aps):
    V = nc.vector
    A = nc.scalar
    T = nc.tensor
    G = nc.gpsimd
    u_ap = aps["u"]; y_ap = aps["y"]
    cf_ap = aps["CF"]; ai_ap = aps["AI"]
    TT = V.tensor_tensor
    GT = G.tensor_tensor
    op = mybir.AluOpType

    # ---------------- pools
    p_cf = ctx.enter_context(tc.tile_pool(name="cf", bufs=1))
    p_ai = ctx.enter_context(tc.tile_pool(name="ai", bufs=1))
    p_uch = ctx.enter_context(tc.tile_pool(name="uch", bufs=7))
    p_apl = ctx.enter_context(tc.tile_pool(name="apl", bufs=24))   # A pair planes bf16
    p_asum = ctx.enter_context(tc.tile_pool(name="asum", bufs=8))  # A pair sums bf16
    p_yf = ctx.enter_context(tc.tile_pool(name="yf", bufs=6))     # Y pair tiles bf16
    p_tmp = ctx.enter_context(tc.tile_pool(name="tmp", bufs=4))    # cm pair temps bf16
    p_flt = ctx.enter_context(tc.tile_pool(name="flt", bufs=1))    # filter tiles
    p_yout = ctx.enter_context(tc.tile_pool(name="yout", bufs=2))
    p_kc = ctx.enter_context(tc.tile_pool(name="kc", bufs=4))
    p_gw = ctx.enter_context(tc.tile_pool(name="gw", bufs=1))
    p_z32 = ctx.enter_context(tc.tile_pool(name="z32", bufs=1))
    p_zp = ctx.enter_context(tc.tile_pool(name="zp", bufs=9))
    p_small = ctx.enter_context(tc.tile_pool(name="small", bufs=1))
    p_gwtmp = ctx.enter_context(tc.tile_pool(name="gwtmp", bufs=1))
    p_ks = ctx.enter_context(tc.tile_pool(name="ks", bufs=1))
    p_ps = ctx.enter_context(tc.tile_pool(name="ps", bufs=6, space="PSUM"))
    p_psk = ctx.enter_context(tc.tile_pool(name="psk", bufs=2, space="PSUM"))

    def fr_(t):
        return t.bitcast(dt.float32r)

    # ---------------- packed parameters: 2 DMAs (HWDGE issue is ~630ns/DMA)
    # PW [128, 128] = [wre2 | wim2] restacked W planes
    # PS2 [33, 512]: rows 0:32 = SEL, row 32 = logdt|D|sgn|Lre|Lim packed
    pw = p_small.tile([128, 256], dt.float32, tag="pw")
    A.dma_start(pw[:], aps["PW"][:])
    psm = p_small.tile([1, 512], dt.float32, tag="psm")
    sel_full = p_small.tile([32, 512], dt.float32, tag="selt")
    wre2 = pw[:, 0:64]
    wim2 = pw[:, 64:128]
    logdt2 = pw[:, 128:192]       # logdt[2c+q] restacked-broadcast
    LreC = pw[:, 192:193]         # Lambda_re[n] column
    LimhC = pw[:, 193:194]        # Lambda_im[n]/2 column
    sel_t = sel_full[:].rearrange("b (al l) -> b al l", al=4)
    logdt = psm[:, 0:128]
    dtile = psm[:, 128:256]
    sgn_row = psm[:, 256:384]
    Lre = psm[:, 384:448]
    Lim_r = psm[:, 448:512]

    # ---------------- constant stationaries: CF (both parities), then AI
    cf_big = {}
    _q = [0]

    def qeng():
        e = (nc.sync, nc.scalar)[_q[0] % 2]
        _q[0] += 1
        return e

    def load_cf(par, cs):
        tl = p_cf.tile([128, NJ, NFT, 128], dt.bfloat16, tag=f"cf{par}{cs}",
                       name=f"cfb{par}_{cs}")
        qeng().dma_start(tl[:], cf_ap[par, cs])
        cf_big[(par, cs)] = tl

    def cf_tile(par, cs, j, ft):
        return cf_big[(par, cs)][:, j, ft, :]

    # u pair-tiles (256 samples each, bf16, pre-transposed in DRAM);
    # sub-chunk cc lives in pair cc//2 slot cc%2
    upairs = {}

    def get_upair(g):
        if g not in upairs:
            t_u = p_uch.tile([128, 2, 4, 128], dt.bfloat16, tag="uch", name=f"up{g}")
            qeng().dma_start(t_u[:], u_ap[g])
            upairs[g] = t_u
        return upairs[g]

    def get_chunk(cc):
        return get_upair(cc // 2)[:, cc % 2]

    load_cf(0, 0)
    get_upair(0)
    load_cf(0, 1)
    get_upair(1)
    nc.sync.dma_start(psm[:], aps["PSM"][:])
    load_cf(1, 0)
    load_cf(1, 1)
    nc.scalar.dma_start(sel_full[:], aps["SELT"][:])
    for g in range(2, 7):
        get_upair(g)

    ai_big = {}
    for cs in range(2):
        tl = p_ai.tile([128, NFT, NLT, 128], dt.bfloat16, tag=f"ai{cs}",
                       name=f"aib{cs}")
        qeng().dma_start(tl[:], ai_ap[cs])
        ai_big[cs] = tl

    def ai_tile(cs, ft, lt):
        return ai_big[cs][:, ft, lt, :]

    # ---------------- forward chunk-DFT: A^c planes (bf16, ft-PAIRED tiles so the
    # pointwise runs [128, 1024] ops), 32 matmuls per chunk
    a_pairs = {}   # (c, cs, fp) -> bf16 [128, 2, 512] covering ft = 2fp, 2fp+1

    def emit_chunk_dft(c):
        par = c % 2
        for ft in range(NFT):
            fp, fi = divmod(ft, 2)
            pss = {}
            for cs in range(2):
                pss[cs] = p_ps.tile([128, 512], dt.float32, tag="ps",
                                    name=f"cdft{c}_{ft}_{cs}")
                if fi == 0:
                    a_pairs[(c, cs, fp)] = p_apl.tile(
                        [128, 2, 512], dt.bfloat16, tag="apl", name=f"A{c}_{cs}_{fp}")
            for j in range(NJ):
                ch = get_chunk(4 * c + j)
                for cs in range(2):
                    T.matmul(pss[cs][:], cf_tile(par, cs, j, ft),
                             ch.rearrange("p b h -> p (b h)"),
                             start=(j == 0), stop=(j == NJ - 1))
            for cs in range(2):
                A.copy(a_pairs[(c, cs, fp)][:, fi, :], pss[cs][:])

    # ---------------- transcendental prologue (restacked [128, 64])
    def horner_exp(dst, x, coefs, eng=V):
        p = dst
        eng.memset(p, float(coefs[0]))
        for c_ in coefs[1:]:
            tq = p_small.tile([x.shape[0], x.shape[1]], dt.float32, tag="horner", bufs=2)
            eng.tensor_tensor(tq[:], p, x, op.mult)
            eng.tensor_scalar_add(p, tq[:], float(c_))

    # dt = exp(logdt2) = (exp(logdt2/8))^8 directly on the restacked [128, 64]
    x8 = p_small.tile([128, 64], dt.float32, tag="x8")
    V.tensor_scalar_mul(x8[:], logdt2, 0.125)
    e8 = p_small.tile([128, 64], dt.float32, tag="e8")
    horner_exp(e8[:], x8[:], EXP8)
    dtv2 = p_small.tile([128, 64], dt.float32, tag="dtv")
    t_a = p_small.tile([128, 64], dt.float32, tag="sq1")
    TT(t_a[:], e8[:], e8[:], op.mult)
    t_b = p_small.tile([128, 64], dt.float32, tag="sq2")
    TT(t_b[:], t_a[:], t_a[:], op.mult)
    TT(dtv2[:], t_b[:], t_b[:], op.mult)

    # -0.5*exp(Lre) column [128, 1] (GPS lane, parallel to the dt chain)
    xl = p_small.tile([128, 1], dt.float32, tag="xl")
    G.tensor_scalar_mul(xl[:], LreC, 0.125)
    el8 = p_small.tile([128, 1], dt.float32, tag="el8")
    horner_exp(el8[:], xl[:], EXP8, eng=G)
    t_c = p_small.tile([128, 1], dt.float32, tag="sq3")
    GT(t_c[:], el8[:], el8[:], op.mult)
    t_d = p_small.tile([128, 1], dt.float32, tag="sq4")
    GT(t_d[:], t_c[:], t_c[:], op.mult)
    negel = p_small.tile([128, 1], dt.float32, tag="negel")
    t_e = p_small.tile([128, 1], dt.float32, tag="sq5")
    GT(t_e[:], t_d[:], t_d[:], op.mult)
    G.tensor_scalar_mul(negel[:], t_e[:], -0.5)

    # ---------------- chunk DFTs 0..3 cover the prologue on PE
    emit_chunk_dft(0)
    emit_chunk_dft(1)

    # D_rep / D_rep_s [128, 128] outer products (independent of the chains);
    # the ones-row ships inside psm (no on-chip memset dependency)
    ones = psm[:, 384:512]
    ps_d = p_psk.tile([128, 256], dt.float32, tag="psk", name="ps_d")
    T.matmul(ps_d[0:128, 0:HS], ones, dtile, start=True, stop=True)
    T.matmul(ps_d[0:128, 128:128 + HS], sgn_row, dtile, start=True, stop=True)
    D_rep = p_small.tile([128, 128], dt.float32, tag="drep")
    A.copy(D_rep[:], ps_d[0:128, 0:HS])
    D_rep_s = p_small.tile([128, 128], dt.float32, tag="dreps")
    A.copy(D_rep_s[:], ps_d[0:128, 128:128 + HS])
    emit_chunk_dft(2)
    emit_chunk_dft(3)

    # ---------------- half-angle pieces on [128, 64]: broadcast columns
    ah = p_small.tile([128, 64], dt.float32, tag="ah")
    TT(ah[:], dtv2[:], negel[:].broadcast_to([128, 64]), op.mult)
    bh = p_small.tile([128, 64], dt.float32, tag="bh")
    GT(bh[:], dtv2[:], LimhC.broadcast_to([128, 64]), op.mult)
    ea = p_small.tile([128, 64], dt.float32, tag="ea")
    horner_exp(ea[:], ah[:], EXP7)
    # sin(bh), cos(bh) via u = bh^2 (sin chain on GPS, cos on DVE)
    ub = p_small.tile([128, 64], dt.float32, tag="ub")
    GT(ub[:], bh[:], bh[:], op.mult)
    sp = p_small.tile([128, 64], dt.float32, tag="sp")
    G.memset(sp[:], float(SIN9[0]))
    for c_ in SIN9[1:]:
        tq = p_small.tile([128, 64], dt.float32, tag="hornerg", bufs=2)
        GT(tq[:], sp[:], ub[:], op.mult)
        G.tensor_scalar_add(sp[:], tq[:], float(c_))
    sb = p_small.tile([128, 64], dt.float32, tag="sb")
    GT(sb[:], sp[:], bh[:], op.mult)          # sin(b/2)
    cp = p_small.tile([128, 64], dt.float32, tag="cp")
    V.memset(cp[:], float(COSC[0]))
    for c_ in COSC[1:]:
        tq = p_small.tile([128, 64], dt.float32, tag="horner", bufs=2)
        TT(tq[:], cp[:], ub[:], op.mult)
        V.tensor_scalar_add(cp[:], tq[:], float(c_))
    cb = p_small.tile([128, 64], dt.float32, tag="cb")
    tq0 = p_small.tile([128, 64], dt.float32, tag="horner", bufs=2)
    TT(tq0[:], cp[:], ub[:], op.mult)
    V.tensor_scalar(cb[:], tq0[:], -1.0, 1.0, op.mult, op.add)   # cos(b/2)

    wre = p_small.tile([128, 64], dt.float32, tag="wre")
    TT(wre[:], ea[:], cb[:], op.mult)         # Re z^(1/2)
    wim = p_small.tile([128, 64], dt.float32, tag="wim")
    GT(wim[:], ea[:], sb[:], op.mult)         # Im z^(1/2)

    # complex squaring: re parts on DVE, im on GPS
    def csq_parts(dre, dim_, sre, sim):
        t1 = p_small.tile([128, 64], dt.float32, tag="csq1", bufs=2)
        TT(t1[:], sre, sre, op.mult)
        t2 = p_small.tile([128, 64], dt.float32, tag="csq2", bufs=2)
        TT(t2[:], sim, sim, op.mult)
        TT(dre, t1[:], t2[:], op.subtract)
        t3 = p_small.tile([128, 64], dt.float32, tag="csq3", bufs=2)
        GT(t3[:], sre, sim, op.mult)
        G.tensor_scalar_mul(dim_, t3[:], 2.0)

    def new_zpair(nm):
        zr = p_zp.tile([128, 64], dt.float32, tag="zp", name=f"{nm}r")
        zi = p_zp.tile([128, 64], dt.float32, tag="zp", name=f"{nm}i")
        return zr, zi

    # ---------------- GW planes [128, 64, 32] holding (Re, -Im) of W z^b
    GWre = p_gw.tile([128, 64, 32], dt.float32, tag="gwre")
    GWim = p_gw.tile([128, 64, 32], dt.float32, tag="gwim")   # stores -Im
    V.tensor_copy(GWre[:, :, 0], wre2)
    V.tensor_scalar_mul(GWim[:, :, 0], wim2, -1.0)

    def cdouble_seg(pre, pim, zr, zi, s0, d0, w, conj_stored, pr=slice(0, 128), co=0):
        # planes [pr, :, co+d0 : co+d0+w] = planes[pr, :, co+s0:+w] * (zr + i zi)
        nhp = pre.shape[1]
        npr = pr.stop - pr.start
        zre = zr[pr].unsqueeze(2).broadcast_to([npr, nhp, w])
        zim = zi[pr].unsqueeze(2).broadcast_to([npr, nhp, w])
        t2 = p_gwtmp.tile([128, 64, 8], dt.float32, tag="gt2", bufs=3)
        t4 = p_gwtmp.tile([128, 64, 8], dt.float32, tag="gt2", bufs=3)
        TT(pre[pr, :, co + d0:co + d0 + w], pre[pr, :, co + s0:co + s0 + w], zre, op.mult)
        GT(t2[pr, 0:nhp, 0:w], pim[pr, :, co + s0:co + s0 + w], zim, op.mult)
        TT(pim[pr, :, co + d0:co + d0 + w], pim[pr, :, co + s0:co + s0 + w], zre, op.mult)
        TT(t4[pr, 0:nhp, 0:w], pre[pr, :, co + s0:co + s0 + w], zim, op.mult)
        TT(pre[pr, :, co + d0:co + d0 + w], pre[pr, :, co + d0:co + d0 + w],
           t2[pr, 0:nhp, 0:w], op.add if conj_stored else op.subtract)
        GT(pim[pr, :, co + d0:co + d0 + w], pim[pr, :, co + d0:co + d0 + w],
           t4[pr, 0:nhp, 0:w], op.subtract if conj_stored else op.add)

    # ---------------- Z planes [128, 64, 32]: cols 0:16 = even-h z^(32a), zeros;
    # cols 16:32 = zeros, odd-h z^(32a).  (zero quadrants gate the pair mode-sum)
    # a=0 cols: 1 in the live quadrant, 0 in the dead one; the doubling chain
    # then propagates the zeros (0 * z = 0), so no large quadrant memsets.
    Zre = p_z32.tile([128, 64, 32], dt.float32, tag="z32re")
    Zim = p_z32.tile([128, 64, 32], dt.float32, tag="z32im")
    V.memset(Zre[0:64, :, 0], 1.0)
    V.memset(Zre[64:128, :, 0], 0.0)
    V.memset(Zim[:, :, 0], 0.0)
    G.memset(Zre[64:128, :, 16], 1.0)
    G.memset(Zre[0:64, :, 16], 0.0)
    G.memset(Zim[:, :, 16], 0.0)

    # interleaved power chain + doubling
    zp = []
    z0 = new_zpair("z0")
    csq_parts(z0[0][:], z0[1][:], wre[:], wim[:])
    zp.append(z0)
    cdouble_seg(GWre[:], GWim[:], zp[0][0][:], zp[0][1][:], 0, 1, 1, True)
    for j in range(1, 5):                     # z^2, z^4, z^8, z^16
        zj = new_zpair(f"z{1 << j}")
        csq_parts(zj[0][:], zj[1][:], zp[-1][0][:], zp[-1][1][:])
        zp.append(zj)
        if j < 4:
            cdouble_seg(GWre[:], GWim[:], zp[j][0][:], zp[j][1][:], 0, 1 << j, 1 << j, True)
    za = []
    z32t = new_zpair("z32")
    csq_parts(z32t[0][:], z32t[1][:], zp[4][0][:], zp[4][1][:])
    za.append(z32t)                           # z^32
    cdouble_seg(GWre[:], GWim[:], zp[4][0][:], zp[4][1][:], 0, 16, 8, True)
    cdouble_seg(GWre[:], GWim[:], zp[4][0][:], zp[4][1][:], 8, 24, 8, True)
    # Z chains: full partitions, both column bases (zeros propagate)
    for co in (0, 16):
        cdouble_seg(Zre[:], Zim[:], za[0][0][:], za[0][1][:], 0, 1, 1, False, co=co)
    for j in range(1, 4):                     # z^64, z^128, z^256
        zj = new_zpair(f"za{j}")
        csq_parts(zj[0][:], zj[1][:], za[-1][0][:], za[-1][1][:])
        za.append(zj)
        for co in (0, 16):
            cdouble_seg(Zre[:], Zim[:], za[j][0][:], za[j][1][:], 0, 1 << j, 1 << j,
                        False, co=co)

    emit_chunk_dft(4)

    # ---------------- channel-paired mode-sum: k[32a+b, h] (128 matmuls)
    # pair hp: stationary GW[:, hp, :] (dense), moving Z[:, hp, :] (zero quadrants)
    # out [32 b, 32 = (16 a-even | 16 a-odd)]
    ks_all = p_ks.tile([32, 64, 32], dt.bfloat16, tag="ksall")
    for qq in range(4):
        kp = p_psk.tile([32, 16, 32], dt.float32, tag="psk", name=f"kp{qq}")
        for i in range(16):
            hp = 16 * qq + i
            T.matmul(kp[0:32, i, :], fr_(GWre[:, hp, :]), fr_(Zre[:, hp, :]),
                     start=True, stop=False)
            T.matmul(kp[0:32, i, :], fr_(GWim[:, hp, :]), fr_(Zim[:, hp, :]),
                     start=False, stop=True)
        V.tensor_copy(ks_all[0:32, 16 * qq:16 * qq + 16, :], kp[:])

    # ---------------- kc tiles [128 l, 128 h] via SEL matmuls (l-major layout)
    # ks_view(c2, al)[b, h] = k[128 c2 + 32 al + b, h]
    ks_v = ks_all[:].rearrange("b hp (par x) -> b hp par x", par=2)
    kc = []
    for c2 in range(4):
        kps = p_psk.tile([128, 128], dt.float32, tag="psk", name=f"kcps{c2}")
        for al in range(4):
            mov = ks_v[:, :, :, 4 * c2 + al].rearrange("b hp par -> b (hp par)")
            T.matmul(kps[:], fr_(sel_t[:, al, :]), mov, start=(al == 0), stop=(al == 3))
        kt = p_kc.tile([128, 128], dt.bfloat16, tag="kc", name=f"kc{c2}")
        V.tensor_copy(kt[:], kps[:])
        kc.append(kt)

    # ---------------- K_f via packed chunk-DFT of k (both parities), computed in
    # filter-consumption order (par=1 first); the filter builds read the psums
    # directly - no SBUF staging
    pks = {}

    def emit_kf(par):
        for cs in range(2):
            pp = p_psk.tile([128, NFT, 128], dt.float32, tag="psk", name=f"kdft{par}{cs}")
            for ft in range(NFT):
                for c2 in range(4):
                    T.matmul(pp[:, ft, :], cf_tile(par, cs, c2, ft), kc[c2][:],
                             start=(c2 == 0), stop=(c2 == 3))
            pks[(par, cs)] = pp

    # ---------------- filter tiles (bf16 [128, 2, 128], ft-paired):
    # variant v=0 (even blocks): K-tilde from pks[par=1] + D_rep_s
    # variant v=1 (odd blocks):  K' from pks[par=0] + D_rep
    fA = {}
    fB = {}
    fD = {}
    for v, par, drep in ((0, 1, D_rep_s), (1, 0, D_rep)):
        emit_kf(par)
        for fp in range(2):
            ta = p_flt.tile([128, 2, 128], dt.bfloat16, tag=f"fA{v}{fp}")
            eng = (V, G)[fp]
            eng.tensor_tensor(ta[:], pks[(par, 0)][:, 2 * fp:2 * fp + 2, :],
                              drep[:].unsqueeze(1).broadcast_to([128, 2, 128]), op.add)
            tb = p_flt.tile([128, 2, 128], dt.bfloat16, tag=f"fB{v}{fp}")
            V.tensor_copy(tb[:], pks[(par, 1)][:, 2 * fp:2 * fp + 2, :])
            fA[(v, fp)] = ta
            fB[(v, fp)] = tb
        # D-tensor pair for fp=0: slot 0 = Nyquist-special, slot 1 = fA[ft=1]
        td = p_flt.tile([128, 2, 128], dt.bfloat16, tag=f"fD{v}")
        eng = (G, V)[v]
        eng.tensor_tensor(td[:, 0, :], pks[(par, 0)][:, 0, :], D_rep[:] if par == 0 else D_rep_s[:], op.add)
        # row 0: packed Nyquist slot: K_nyq + D (no sign flip: (-1)^512 = +1)
        TT(td[0:1, 0, :], pks[(par, 1)][0:1, 0, :], D_rep[0:1, :], op.add)
        V.tensor_copy(td[:, 1, :], fA[(v, 0)][:, 1, :])
        fD[v] = td
        V.memset(fB[(v, 0)][0:1, 0, :], 0.0)  # Im slot for f=0/Nyquist is zero

    # gate column: data-dependent on the last filter tile so the list scheduler
    # cannot hoist the block adds into the prologue-critical engine windows
    gate = p_small.tile([128, 1], dt.float32, tag="gate")
    V.tensor_scalar(gate[:], fD[1][:, 0, 0:1], 0.0, 1.0, op.mult, op.add)

    emit_chunk_dft(5)

    # ---------------- main loop (paired [128, 1024] pointwise ops)
    def kb2(ap3):
        # [128, 2, 128] filter -> broadcast over the 4 batch groups
        return ap3.unsqueeze(2).broadcast_to([128, 2, 4, 128])

    def r4(t):
        return t[:].rearrange("p fp (b h) -> p fp b h", b=4)

    for blk in range(NBLK):
        v = blk % 2
        yr_p, yi_p = [], []
        for fp in range(2):
            ac_cur = a_pairs[(blk, 0, fp)]
            as_cur = a_pairs[(blk, 1, fp)]
            if blk == 0:
                xc, xs = ac_cur, as_cur
            else:
                ac_prev = a_pairs[(blk - 1, 0, fp)]
                as_prev = a_pairs[(blk - 1, 1, fp)]
                xc = p_asum.tile([128, 2, 512], dt.bfloat16, tag="asum",
                                 name=f"xc{blk}_{fp}")
                G.scalar_tensor_tensor(xc[:], ac_cur[:], gate[:], ac_prev[:],
                                       op.mult, op.add)
                xs = p_asum.tile([128, 2, 512], dt.bfloat16, tag="asum",
                                 name=f"xs{blk}_{fp}")
                G.scalar_tensor_tensor(xs[:], as_cur[:], gate[:], as_prev[:],
                                       op.mult, op.add)
            # complex multiply by the parity filter (DVE, bf16)
            t1 = p_tmp.tile([128, 2, 512], dt.bfloat16, tag="t1")
            TT(r4(t1), r4(xc), kb2(fA[(v, fp)][:]), op.mult)
            t2 = p_tmp.tile([128, 2, 512], dt.bfloat16, tag="t2")
            TT(r4(t2), r4(xs), kb2(fB[(v, fp)][:]), op.mult)
            yr = p_yf.tile([128, 2, 512], dt.bfloat16, tag="yf", name=f"yr{blk}_{fp}")
            TT(yr[:], t1[:], t2[:], op.subtract)
            t3 = p_tmp.tile([128, 2, 512], dt.bfloat16, tag="t1")
            TT(r4(t3), r4(xc), kb2(fB[(v, fp)][:]), op.mult)
            dten = fD[v] if fp == 0 else fA[(v, fp)]
            t4 = p_tmp.tile([128, 2, 512], dt.bfloat16, tag="t2")
            TT(r4(t4), r4(xs), kb2(dten[:]), op.mult)
            yi = p_yf.tile([128, 2, 512], dt.bfloat16, tag="yf", name=f"yi{blk}_{fp}")
            TT(yi[:], t3[:], t4[:], op.add)
            yr_p.append(yr)
            yi_p.append(yi)
        if blk + 6 < NBLK:
            emit_chunk_dft(blk + 6)
        yo = p_yout.tile([128, NLT, 512], dt.bfloat16, tag="yout")
        for lt in range(NLT):
            py = p_ps.tile([128, 512], dt.float32, tag="ps", name=f"py{blk}_{lt}")
            for ft in range(NFT):
                T.matmul(py[:], ai_tile(0, ft, lt), yr_p[ft // 2][:, ft % 2, :],
                         start=(ft == 0), stop=False)
                T.matmul(py[:], ai_tile(1, ft, lt), yi_p[ft // 2][:, ft % 2, :],
                         start=False, stop=(ft == NFT - 1))
            A.copy(yo[:, lt, :], py[:])
            if blk >= NBLK - 2:   # drain the tail per l-tile
                eng = (nc.sync, nc.scalar)[lt % 2]
                eng.dma_start(y_ap[blk, :, lt].unsqueeze(1),
                              yo[:, lt:lt + 1, :].rearrange("p lt (b h) -> p lt b h", b=4))
        if blk < NBLK - 2:
            eng = nc.sync if blk % 2 == 0 else nc.scalar
            eng.dma_start(y_ap[blk], yo[:].rearrange("p lt (b h) -> p lt b h", b=4))


def _build_program():
    if _prog.built:
        return
    nc = bacc.Bacc("TRN2", target_bir_lowering=False, debug=False,
                   num_devices=NCORES)
    aps = {}
    aps["u"] = nc.dram_tensor("u", [NCH // 2, 128, 2, 4, HS], dt.bfloat16,
                              kind="ExternalInput").ap()
    aps["PW"] = nc.dram_tensor("PW", [128, 256], dt.float32, kind="ExternalInput").ap()
    aps["PSM"] = nc.dram_tensor("PSM", [1, 512], dt.float32, kind="ExternalInput").ap()
    aps["SELT"] = nc.dram_tensor("SELT", [32, 512], dt.float32, kind="ExternalInput").ap()
    aps["CF"] = nc.dram_tensor("CF", [2, 2, 128, NJ, NFT, 128], dt.bfloat16,
                               kind="ExternalInput").ap()
    aps["AI"] = nc.dram_tensor("AI", [2, 128, NFT, NLT, 128], dt.bfloat16,
                               kind="ExternalInput").ap()
    aps["y"] = nc.dram_tensor("y", [NBLK, 128, NLT, 4, HS], dt.bfloat16,
                              kind="ExternalOutput").ap()
    with tile.TileContext(nc, trace_sim=False) as tc:
        with ExitStack() as ctx:
            _emit_kernel(nc, tc, ctx, aps)
    nc.compile()
    _prog.nc = nc
    _prog.CF, _prog.AI, _prog.SGN, _prog.SEL = build_constants()
    _prog.built = True


def make_in_maps(u, D, log_dt, W_re, W_im, Lambda_re, Lambda_im):
    import ml_dtypes
    bf16 = ml_dtypes.bfloat16
    _build_program()
    # u [B, L, H] -> per-core pre-transposed bf16 [NCH/2, 128, 2, 4, HS]
    u_t = np.ascontiguousarray(
        u.reshape(B, NCH // 2, 2, 128, H).transpose(1, 3, 2, 0, 4)).astype(bf16)
    in_maps = []
    for c in range(NCORES):
        h0 = c * HS
        # restack W [HS, N] -> [128=(n, h%2), 64=h//2], packed with wim
        wre_s = W_re[h0:h0 + HS]
        wim_s = W_im[h0:h0 + HS]
        pw = np.zeros((128, 256), f32)
        for q in range(2):
            pw[64 * q:64 * q + 64, 0:64] = wre_s[q::2, :].T
            pw[64 * q:64 * q + 64, 64:128] = wim_s[q::2, :].T
            # logdt restacked-broadcast: same row of 64 values on every n-row
            pw[64 * q:64 * q + 64, 128:192] = log_dt[h0:h0 + HS][q::2][None, :]
            pw[64 * q:64 * q + 64, 192] = Lambda_re
            pw[64 * q:64 * q + 64, 193] = Lambda_im * 0.5
        psm = np.zeros((1, 512), f32)
        psm[0, 0:128] = log_dt[h0:h0 + HS]
        psm[0, 128:256] = D[h0:h0 + HS]
        psm[0, 256:384] = _prog.SGN[0]
        psm[0, 384:512] = 1.0
        in_maps.append({
            "u": np.ascontiguousarray(u_t[:, :, :, :, h0:h0 + HS]),
            "PW": pw,
            "PSM": psm,
            "SELT": _prog.SEL.reshape(32, 512),
            "CF": _prog.CF,
            "AI": _prog.AI,
        })
    return in_maps


LAST_RESULTS = None


def kernel(u, D, Lambda_re, Lambda_im, log_dt, W_re, W_im):
    global LAST_RESULTS
    from concourse.bass_utils import run_bass_kernel_spmd
    in_maps = make_in_maps(u, D, log_dt, W_re, W_im, Lambda_re, Lambda_im)
    res = run_bass_kernel_spmd(_prog.nc, in_maps, core_ids=list(range(NCORES)))
    LAST_RESULTS = res
    # y_core [NBLK, 128, NLT, 4, HS] (blk, p, lt, b, h) -> [B, L, HS]
    ys = []
    for c in range(NCORES):
        yc = np.asarray(res.results[c]["y"], dtype=np.float32)
        ys.append(yc.transpose(3, 0, 2, 1, 4).reshape(B, L, HS))
    return np.concatenate(ys, axis=2)
